# revision 42
# baseline (speedup 1.0000x reference)
"""Trainium2 Bass kernel for nn_Attention_18305150616358.

Dense transformer attention block with an LMF (low-rank multimodal fusion)
modulation applied to the query. Sharding: 8 cores = 2 batches x 4 head
groups (3 heads each). The LMF is algebraically folded on the host into a
per-batch effective query weight:

    text_f = q @ Wt + ct'          (Wt = sum_r lmf_text_w[r], affine)
    lat_f  = [latent,1] @ Wl + cl' (per batch row vector)
    q_eff  = (x @ Wq + bq) @ Wt * lat_f = x @ (Wq@Wt * lat_f) + b_eff

so each core runs a plain causal attention over its 3 heads and writes the
partial (row-slice of c_proj) output projection; the host sums the 4
partials per batch and adds c_proj_b.

v6 design notes (vs the 130us v5):
  * PE warm-up: a stream of identity matmuls issued at t~=0 keeps the PE
    HAM un-throttled through the input-DMA wait, so the first real qkv
    matmuls start ~3.5us in at the full 2.4GHz clock (was: first matmul
    at 12.4us, half clock until 21us).
  * input DMAs consolidated to one descriptor per tensor (3D APs) -- the
    0.65us/instr descriptor generation was serializing startup.
  * V is produced directly in [keys, dims] layout by flipping the matmul
    orientation (stationary = x position-tile, moving = Wv slab): kills
    the 16 PE transposes, 32 DVE copies and the half-empty jm4 qkv slot.
    The v bias is folded into c_proj_b on the host (exact).
  * softmax reciprocal moved off ACT onto the DVE custom op
    reciprocal_approx_fast (~18 correct bits); ACT now only runs the
    exps, which is what bounds attention(3).
  * guard memsets moved from DVE to GpSimd.
  * fp16 everywhere on the PE (fp8 DoubleRow was evaluated: each e4m3
    quantization stage alone costs ~2-3e-2 max-rel error vs the 2e-2
    budget -- dead end).
"""

import os
import sys

for _p in ("/opt/trn_rl_repo", "/opt/pypackages"):
    if os.path.isdir(_p) and _p not in sys.path:
        sys.path.insert(0, _p)

import numpy as np

S = 2048
D = 768
NH = 12
HD = 64
HPC = 3  # heads per core
N_CORES = 8
QC = 512  # q chunk (moving free dim)
NQC = S // QC  # 4
KT = 128  # key tile

# 64-wide column slots of the fused per-core q/k weight (768, 384):
SLOTS = [
    ("q", 0), ("q", 1),
    ("k", 0), ("k", 1),
    ("q", 2), ("k", 2),
]
K_CH = {0: 0, 1: 0, 2: 1}   # head -> qkvT_k chunk (rows: l0 0:64, l1/l2 64:128)

_CACHE = {}


def _build_program():
    import concourse.bass as bass
    from concourse import bacc, mybir
    from concourse.tile import TileContext
    from concourse.hw_specs import get_activation_tables

    f32 = mybir.dt.float32
    fp16 = mybir.dt.float16

    nc = bacc.Bacc("TRN2", target_bir_lowering=False, debug=False,
                   num_devices=N_CORES)

    x_d = nc.dram_tensor("xT", (D, S), fp16, kind="ExternalInput").ap()
    w_d = nc.dram_tensor("w_qkv", (D, 384), fp16, kind="ExternalInput").ap()
    wv_d = nc.dram_tensor("w_v", (D, 192), fp16, kind="ExternalInput").ap()
    b_d = nc.dram_tensor("b_qkv", (384,), f32, kind="ExternalInput").ap()
    wp_d = nc.dram_tensor("w_proj", (192, D), fp16, kind="ExternalInput").ap()
    out_d = nc.dram_tensor("out_partial", (S, D), fp16,
                           kind="ExternalOutput").ap()

    from contextlib import ExitStack

    with TileContext(nc) as tc, ExitStack() as ctx:
        # Pre-load the Exp activation table so the act-table pass never
        # inserts a swap mid-kernel.
        tabs = list(get_activation_tables(nc.m.arch).keys())
        nc.scalar.add_instruction(mybir.InstLoadActFuncSet(
            name=nc.get_next_instruction_name(),
            act_func_set_id=tabs.index("natural_log_exp_and_others"),
            ins=[], outs=[]))

        singles = ctx.enter_context(tc.tile_pool(name="singles", bufs=1))
        exp_pool = ctx.enter_context(tc.tile_pool(name="expT", bufs=4))
        rec_pool = ctx.enter_context(tc.tile_pool(name="rec", bufs=3))
        osb_pool = ctx.enter_context(tc.tile_pool(name="osb", bufs=4))
        sc_psum = ctx.enter_context(tc.tile_pool(name="sc_ps", bufs=2, space="PSUM"))
        pv_psum = ctx.enter_context(tc.tile_pool(name="pv_ps", bufs=2, space="PSUM"))
        mm_psum = ctx.enter_context(tc.tile_pool(name="mm_ps", bufs=2, space="PSUM"))

        w_sb = singles.tile([128, 6, 384], fp16)    # [jp slab, q/k slots]
        wv_sb = singles.tile([128, 6, 192], fp16)   # [jp slab, v slots]
        bias_sb = singles.tile([128, 3], f32)
        wp_sb = singles.tile([128, 2, D], fp16)
        xT = singles.tile([128, 6, S], fp16)

        # pad rows 64:128 of wp slab 1 with zeros: the proj's second
        # matmul then contracts a full 128 partitions (sub-128 contractions
        # run the PE at the unwarmed half clock)
        nc.vector.memset(wp_sb[64:128, 1, :], 0.0)

        # fp16 identity: now only PE warm-up fodder
        id16 = singles.tile([128, 128], fp16)
        nc.gpsimd.memset(id16, 0.0)
        nc.gpsimd.affine_select(
            out=id16, in_=id16, compare_op=mybir.AluOpType.not_equal,
            fill=1.0, base=0, pattern=[[-1, 128]], channel_multiplier=1)

        qkvT_k = singles.tile([128, 2, S], fp16)  # ch0 [k0|k1], ch1 [junk|k2]
        qk2 = singles.tile([128, 3, S], fp16)
        aT = singles.tile([128, 2, S], fp16)
        # [V_l | ones] stationaries: [128 keys, G, slab, (64 V|64 ones) x 3]
        vones = singles.tile([128, S // KT // 2, 2, HPC, 2, 64], fp16)

        # zero the padded/junk halves (NaN guards for the stationary
        # reads) on GpSimd, split per chunk so chunk c's pieces ride
        # the fill stream of attention(c-1).
        def emit_guards(c):
            cs = slice(c * QC, (c + 1) * QC)
            nc.gpsimd.memset(aT[64:128, 1, cs], 0.0)
            nc.gpsimd.memset(qkvT_k[0:64, 1, cs], 0.0)
            nc.gpsimd.memset(qk2[64:128, 0, cs], 0.0)
            nc.gpsimd.memset(qk2[0:64, 1, cs], 0.0)
            nc.gpsimd.memset(qk2[0:64, 2, cs], 0.0)

        def at_slice(l, fs):
            if l == 0:
                return aT[0:64, 0, fs]
            if l == 1:
                return aT[64:128, 0, fs]
            return aT[0:64, 1, fs]

        # Input DMAs. The first qkv group needs w + x chunk0: those are
        # split in jp halves so the first LDWEIGHTS can go as soon as the
        # first half lands. Small/late-needed tensors (bias, wp) ride the
        # GpSimd software-DMA path so their tiny packets never clog the
        # queues carrying w/x (a leading bias DMA cost ~5us of startup in
        # v6.0: 384 4-byte packets at the head of the SP queue).
        w_src = w_d.rearrange("(j p) e -> p j e", p=128)
        x_src = x_d.rearrange("(j p) s -> p j s", p=128)
        nc.sync.dma_start(out=w_sb[:, 0:3, :], in_=w_src[:, 0:3, :])
        nc.scalar.dma_start(out=xT[:, 0:3, 0:QC], in_=x_src[:, 0:3, 0:QC])
        nc.sync.dma_start(out=w_sb[:, 3:6, :], in_=w_src[:, 3:6, :])
        nc.scalar.dma_start(out=xT[:, 3:6, 0:QC], in_=x_src[:, 3:6, 0:QC])
        nc.gpsimd.dma_start(out=bias_sb,
                            in_=b_d.rearrange("(c p) -> p c", p=128))
        nc.sync.dma_start(out=wv_sb, in_=wv_d.rearrange("(j p) e -> p j e", p=128))
        nc.gpsimd.dma_start(out=wp_sb[:, 0, :], in_=wp_d[0:128, :])
        nc.gpsimd.dma_start(out=wp_sb[0:64, 1, :], in_=wp_d[128:192, :])
        # x chunk 1 separately: qkv(1) runs as attention(0) fill inside
        # the HAM warm-up window -- the earlier it can go, the denser the
        # cold region and the sooner the clock un-throttles.
        nc.scalar.dma_start(out=xT[:, :, QC:2 * QC], in_=x_src[:, :, QC:2 * QC])
        nc.scalar.dma_start(out=xT[:, :, 2 * QC:S], in_=x_src[:, :, 2 * QC:S])

        # PE warm-up: junk matmuls on the identity tile keep the HAM
        # activity window hot while the input DMAs land (first real matmul
        # can go at ~12.5us: preamble ~7us + w/x first halves ~5us), so
        # real matmuls start at the full 2.4GHz clock. The PE queue is
        # in-order, so these must ALL come before the first real matmul.
        wu_state = {"n": 0}

        def emit_wu(n=1):
            for _ in range(n):
                wps = mm_psum.tile([128, 128], f32, tag="mm",
                                   name=f"wu{wu_state['n']}")
                wu_state["n"] += 1
                nc.tensor.matmul(wps, id16, id16, start=True, stop=True)

        emit_wu(20)

        # ---- qkv q/k: one jm group (128 out cols, 6 contraction matmuls) ----
        def emit_qkv_jm(c, jm):
            cs = slice(c * QC, (c + 1) * QC)
            ps = mm_psum.tile([128, QC], f32, tag="mm")
            for jp in range(6):
                nc.tensor.matmul(
                    ps,
                    w_sb[:, jp, jm * 128:(jm + 1) * 128],
                    xT[:, jp, cs],
                    start=(jp == 0), stop=(jp == 5))
            if jm == 0:      # [q0 | q1]
                nc.vector.tensor_scalar_add(
                    out=qk2[0:64, 0, cs], in0=ps[0:64, :],
                    scalar1=bias_sb[0:64, 0:1])
                nc.vector.tensor_scalar_add(
                    out=qk2[64:128, 1, cs], in0=ps[64:128, :],
                    scalar1=bias_sb[64:128, 0:1])
            elif jm == 1:    # [k0 | k1] -> one merged 128-row eviction
                nc.vector.tensor_scalar_add(
                    out=qkvT_k[:, 0, cs], in0=ps,
                    scalar1=bias_sb[:, 1:2])
            else:            # [q2 | k2]; q2 shifts partitions
                nc.vector.tensor_scalar_add(
                    out=qk2[64:128, 2, cs], in0=ps[0:64, :],
                    scalar1=bias_sb[0:64, 2:3])
                nc.vector.tensor_scalar_add(
                    out=qkvT_k[64:128, 1, cs], in0=ps[64:128, :],
                    scalar1=bias_sb[64:128, 2:3])

        # ---- V for one key subtile (128 keys), flipped orientation ----
        # stationary = x position-tile, moving = Wv slab: psum comes out
        # as [128 positions(keys), 192 v-dims] and is evicted straight
        # into the vones layout (v bias folded into c_proj_b on host).
        def emit_vm(st):
            G, sl = st // 2, st % 2
            ps = mm_psum.tile([128, 192], f32, tag="mm")
            for jp in range(6):
                nc.tensor.matmul(
                    ps,
                    xT[:, jp, st * 128:(st + 1) * 128],
                    wv_sb[:, jp, :],
                    start=(jp == 0), stop=(jp == 5))
            nc.vector.tensor_copy(
                out=vones[:, G, sl, :, 0, :],
                in_=ps.rearrange("p (l v) -> p l v", v=64))

        # ---- attention for chunk c, with PE filler interleaved ----
        def emit_attention(c, fill_one):
            for l in range(HPC):
                kch = K_CH[l]
                pv = pv_psum.tile([128, QC], f32, tag="pv")
                n_groups = 2 * (c + 1)

                def q0_of(kt):
                    return max(0, 128 * (kt - 4 * c))

                def emit_qk(G):
                    sc = sc_psum.tile([128, 2, QC], f32, tag="sc",
                                      name=f"sc_{c}_{l}_{G}")
                    for jj in range(2):
                        kt = 2 * G + jj
                        q0 = q0_of(kt)
                        nc.tensor.matmul(
                            sc[:, jj, q0:QC],
                            qkvT_k[:, kch, kt * 128:(kt + 1) * 128],
                            qk2[:, l, c * QC + q0:(c + 1) * QC],
                            start=True, stop=True)
                    return sc

                def emit_exp_pv(G, sc):
                    diag = G >= 2 * c
                    q0e = q0_of(2 * G)
                    expT = exp_pool.tile([128, 2, QC], fp16, tag="expT",
                                         name=f"expT_{c}_{l}_{G}")
                    nc.scalar.activation(
                        out=expT[:, :, q0e:QC],
                        in_=sc[:, :, q0e:QC],
                        func=mybir.ActivationFunctionType.Exp,
                        scale=1.0 / np.sqrt(np.float32(HD)))
                    if diag:
                        # causal fill on the two diagonal-region slabs:
                        # even slab: keep where q >= key  (1 + t - p > 0)
                        nc.gpsimd.affine_select(
                            out=expT[:, 0, q0e:q0e + 128],
                            in_=expT[:, 0, q0e:q0e + 128],
                            compare_op=mybir.AluOpType.is_gt,
                            fill=0.0, base=1, pattern=[[1, 128]],
                            channel_multiplier=-1)
                        # odd slab: keep where q-128 >= key (t - 127 - p > 0);
                        # also zero-fills the [q0e, q0e+128) stale region
                        nc.gpsimd.affine_select(
                            out=expT[:, 1, q0e:q0e + 256],
                            in_=expT[:, 1, q0e:q0e + 256],
                            compare_op=mybir.AluOpType.is_gt,
                            fill=0.0, base=-127, pattern=[[1, 256]],
                            channel_multiplier=-1)
                    for jj in range(2):
                        kt = 2 * G + jj
                        q0 = q0_of(kt)
                        nc.tensor.matmul(
                            pv[:, q0:QC],
                            vones[:, G, jj, l, :, :].rearrange(
                                "p a b -> p (a b)"),
                            expT[:, jj, q0:QC],
                            start=(G == 0 and jj == 0),
                            stop=(G == n_groups - 1 and jj == 1))

                # one-deep software pipeline: QK(G+1) before exp/PV(G)
                prev = emit_qk(0)
                for G in range(1, n_groups):
                    sc = emit_qk(G)
                    emit_exp_pv(G - 1, prev)
                    fill_one()
                    prev = sc
                emit_exp_pv(n_groups - 1, prev)
                fill_one()
                # normalize, all on the DVE (measured HW constraints: the
                # custom recip op only works at base partition 0 for BOTH
                # operands; dual-input DVE ops need equal input bases;
                # single-input DVE ops may shift partitions freely):
                #   copy den (pv[64:128]) down to base 0, recip, mul.
                # The very last head's normalize gates the proj tail, so
                # it runs in two column halves: proj(12,13) unblock after
                # the first half's mul.
                rec = rec_pool.tile([128, QC], f32, tag="rec")
                rec2 = rec_pool.tile([128, QC], f32, tag="rec")
                halves = 2 if (c == NQC - 1 and l == HPC - 1) else 1
                hw = QC // halves
                for h in range(halves):
                    hs = slice(h * hw, (h + 1) * hw)
                    nc.vector.tensor_copy(out=rec[0:64, hs],
                                          in_=pv[64:128, hs])
                    nc.vector.reciprocal_approx_fast(
                        out=rec2[0:64, hs], in_=rec[0:64, hs])
                    nc.vector.tensor_mul(
                        out=at_slice(l, slice(c * QC + h * hw,
                                              c * QC + (h + 1) * hw)),
                        in0=pv[0:64, hs], in1=rec2[0:64, hs])

        # split-phase proj for the very last attention chunk: phase A (the
        # heads-0/1 matmuls, independent of head 2) runs in the final fill
        # slot while head 2's softmax normalize chain drains; phase B
        # finishes after the aT write.
        proj_hold = {}

        def emit_proj_phaseA(st, pool=None, tag="mm"):
            # pool override: the last phaseA's go into the sc psum pool,
            # which is idle once attention(3)'s final QK has run -- the
            # held tiles then don't block the mm rotation.
            pool = pool or mm_psum
            pos = []
            for nch in range(2):
                po = pool.tile([128, 384], f32, tag=tag,
                               name=f"poA_{st}_{nch}")
                nc.tensor.matmul(
                    po,
                    aT[:, 0, st * 128:(st + 1) * 128],
                    wp_sb[:, 0, nch * 384:(nch + 1) * 384],
                    start=True, stop=False)
                pos.append(po)
            proj_hold[st] = pos

        def emit_proj_phaseB(st):
            osb = osb_pool.tile([128, D], fp16, tag="osb")
            for nch, po in enumerate(proj_hold.pop(st)):
                nc.tensor.matmul(
                    po,
                    aT[:, 1, st * 128:(st + 1) * 128],
                    wp_sb[:, 1, nch * 384:(nch + 1) * 384],
                    start=False, stop=True)
                if nch == 0:
                    nc.scalar.copy(
                        out=osb[:, nch * 384:(nch + 1) * 384], in_=po)
                else:
                    nc.vector.tensor_copy(
                        out=osb[:, nch * 384:(nch + 1) * 384], in_=po)
                eng = nc.scalar if nch == 0 else nc.sync
                eng.dma_start(
                    out=out_d[st * 128:(st + 1) * 128,
                              nch * 384:(nch + 1) * 384],
                    in_=osb[:, nch * 384:(nch + 1) * 384])

        def emit_proj_st(st, tail=False):
            osb = osb_pool.tile([128, D], fp16, tag="osb")
            for nch in range(2):
                # at the tail, alternate po tiles between the mm and pv
                # psum pools (pv is done by then): 4 in-flight po tiles
                # instead of 2, so the matmul->evict->matmul ladder
                # pipelines instead of serializing on bank reuse.
                pool = pv_psum if (tail and nch == 1) else mm_psum
                po = pool.tile([128, 384], f32, tag="pv" if pool is pv_psum
                               else "mm")
                nc.tensor.matmul(
                    po,
                    aT[:, 0, st * 128:(st + 1) * 128],
                    wp_sb[:, 0, nch * 384:(nch + 1) * 384],
                    start=True, stop=False)
                nc.tensor.matmul(
                    po,
                    aT[:, 1, st * 128:(st + 1) * 128],
                    wp_sb[:, 1, nch * 384:(nch + 1) * 384],
                    start=False, stop=True)
                if tail and nch == 0:
                    # ACT is idle after the last exp: halve the tail's
                    # serial eviction chain
                    nc.scalar.copy(
                        out=osb[:, nch * 384:(nch + 1) * 384], in_=po)
                else:
                    nc.vector.tensor_copy(
                        out=osb[:, nch * 384:(nch + 1) * 384], in_=po)
                if tail:
                    # per-half DMA so the final transfer starts as soon as
                    # its half is evicted; alternate issue queues (the
                    # 0.6us descriptor generation otherwise serializes the
                    # last transfers on one queue)
                    eng = nc.scalar if nch == 0 else nc.sync
                    eng.dma_start(
                        out=out_d[st * 128:(st + 1) * 128,
                                  nch * 384:(nch + 1) * 384],
                        in_=osb[:, nch * 384:(nch + 1) * 384])
            if not tail:
                nc.sync.dma_start(out=out_d[st * 128:(st + 1) * 128, :],
                                  in_=osb)

        # ---- main schedule ----
        # Fill inventory per attention chunk. attention(3) has the largest
        # exp load (the PE idles waiting on the ACT engine there), so every
        # fill whose data dependencies allow it is deferred into att(3):
        # only the q evictions of qkv(3) (jm 0, 2) must precede att(3)
        # (its first QK reads chunk-3 queries); k/v of chunk 3 are first
        # read at group 6+, after the early fill slots have run.
        emit_guards(0)
        # vones "ones" halves: G0/G1 are read by attention(0); later G's
        # ride the fill streams (gpsimd queue order matters -- guards(0)
        # gate attention(0)'s first QK, so they go first).
        for G in range(2):
            nc.gpsimd.memset(vones[:, G, :, :, 1, :], 1.0)
        for jm in range(3):
            emit_qkv_jm(0, jm)
        # only vm0/vm1 must precede attention(0)'s first PV; vm2/vm3 ride
        # the fill stream (eagerly, before PV group 1), so att(0)'s first
        # QK/exp start ~3us earlier
        for st in range(2):
            emit_vm(st)

        # NOTE emission-order constraint: attention(c) reads vones tiles up
        # to st=4c+3, so vm(st) must be EMITTED before the attention chunk
        # that reads it (or eagerly at its start, for chunk 3).
        fill_plan = {
            0: [("v", 2), ("v", 3)] + [(1, jm) for jm in range(3)]
               + [("g", 1), ("o", 2), ("o", 3)]
               + [("v", st) for st in range(4, 8)],
            1: [("g", 2)] + [("o", G) for G in range(4, 8)]
               + [(2, jm) for jm in range(3)]
               + [("v", st) for st in range(8, 12)]
               + [("p", st) for st in range(0, 4)],
            2: [("g", 3), (3, 0), (3, 2)]
               + [("p", st) for st in range(4, 8)],
            3: [(3, 1)]
               + [("v", st) for st in range(12, 16)]
               + [("p", st) for st in range(8, 12)]
               + [("pA", 12), ("pAsc", 13)],
        }
        # chunk 3's qkv/v fills feed attention(3) itself (its groups 6+
        # read chunk-3 K/V): they must stay eager; only proj is paced.
        eager_n = {0: 5, 1: 0, 2: 0, 3: 5}

        def run_fill(f):
            if f[0] == "pAsc":
                emit_proj_phaseA(f[1], pool=sc_psum, tag="sc")
            elif f[0] == "pA":
                emit_proj_phaseA(f[1])
            elif f[0] == "v":
                emit_vm(f[1])
            elif f[0] == "p":
                emit_proj_st(f[1])
            elif f[0] == "g":
                emit_guards(f[1])
            elif f[0] == "o":
                nc.gpsimd.memset(vones[:, f[1], :, :, 1, :], 1.0)
            else:
                emit_qkv_jm(f[0], f[1])

        for c in range(NQC):
            fills = list(fill_plan[c])
            n_slots = HPC * 2 * (c + 1)
            state = {"slot": 0, "done": 0}
            ne = eager_n[c]

            def fill_one():
                state["slot"] += 1
                want = max(ne, len(fill_plan[c]) * state["slot"] // n_slots)
                while fills and state["done"] < want:
                    run_fill(fills.pop(0))
                    state["done"] += 1

            emit_attention(c, fill_one)
            while fills:
                run_fill(fills.pop(0))
        emit_proj_phaseB(12)
        emit_proj_phaseB(13)
        for st in range(14, 16):
            emit_proj_st(st, tail=True)

    nc.compile()
    return nc


def _fold_inputs(x, latent_syntax, c_attn_w, c_attn_b, c_proj_w, c_proj_b,
                 lmf_text_w, lmf_text_b, lmf_lat_w, lmf_lat_b):
    """Host-side algebraic folding of the LMF into per-core weights."""
    f = np.float32
    x = np.ascontiguousarray(x, dtype=f)
    B = x.shape[0]
    Wq, Wk, Wv = (c_attn_w[:, :D], c_attn_w[:, D:2 * D], c_attn_w[:, 2 * D:])
    bq, bk, bv = (c_attn_b[:D], c_attn_b[D:2 * D], c_attn_b[2 * D:])
    Wt = lmf_text_w.sum(0).astype(f)       # (D+1, D)
    ct = lmf_text_b.sum(0).astype(f)
    Wl = lmf_lat_w.sum(0).astype(f)
    cl = lmf_lat_b.sum(0).astype(f)
    W_text = (Wq.astype(f) @ Wt[:D])       # (D, D)
    b_text = bq.astype(f) @ Wt[:D] + Wt[D] + ct
    lat = latent_syntax[:, 0, :].astype(f)
    lat1 = np.concatenate([lat, np.ones((B, 1), f)], axis=-1)
    lat_f = lat1 @ Wl + cl                 # (B, D)

    in_maps = []
    for core in range(N_CORES):
        b = core // 4
        g = core % 4
        Wq_eff = W_text * lat_f[b][None, :]
        bq_eff = b_text * lat_f[b]
        mats = {"q": Wq_eff, "k": Wk.astype(f)}
        vecs = {"q": bq_eff, "k": bk.astype(f)}
        W_core = np.empty((D, 384), f)
        b_core = np.zeros((384,), f)
        for slot, (kind, l) in enumerate(SLOTS):
            h = 3 * g + l
            W_core[:, slot * 64:(slot + 1) * 64] = \
                mats[kind][:, h * 64:(h + 1) * 64]
            b_core[slot * 64:(slot + 1) * 64] = vecs[kind][h * 64:(h + 1) * 64]
        in_maps.append({
            "xT": np.ascontiguousarray(x[b].T.astype(np.float16)),
            "w_qkv": np.ascontiguousarray(W_core.astype(np.float16)),
            "w_v": np.ascontiguousarray(
                Wv[:, 192 * g:192 * (g + 1)].astype(np.float16)),
            "b_qkv": b_core,
            "w_proj": np.ascontiguousarray(
                c_proj_w[192 * g:192 * (g + 1), :].astype(np.float16)),
        })
    return in_maps


def _get_program():
    if "nc" not in _CACHE:
        _CACHE["nc"] = _build_program()
    return _CACHE["nc"]


def kernel(**inputs):
    from concourse import bass_utils

    nc = _get_program()
    in_maps = _fold_inputs(**inputs)
    res = bass_utils.run_bass_kernel_spmd(nc, in_maps,
                                          core_ids=list(range(N_CORES)))
    B = inputs["x"].shape[0]
    # v bias folded here: a_h += bv_h for every position, so the proj
    # output gains the constant row bv @ c_proj_w (exact).
    bv = inputs["c_attn_b"][2 * D:].astype(np.float32)
    cpb = inputs["c_proj_b"].astype(np.float32) + \
        bv @ inputs["c_proj_w"].astype(np.float32)
    out = np.zeros((B, S, D), np.float32)
    for b in range(B):
        acc = np.zeros((S, D), np.float32)
        for g in range(4):
            acc += res.results[4 * b + g]["out_partial"].astype(np.float32)
        out[b] = acc + cpb[None, :]
    return out


# revision 43
# speedup vs baseline: 1.0499x; 1.0499x over previous
"""Trainium2 Bass kernel for nn_Attention_18305150616358.

Dense transformer attention block with an LMF (low-rank multimodal fusion)
modulation applied to the query. Sharding: 8 cores = 2 batches x 4 head
groups (3 heads each). The LMF is algebraically folded on the host into a
per-batch effective query weight:

    text_f = q @ Wt + ct'          (Wt = sum_r lmf_text_w[r], affine)
    lat_f  = [latent,1] @ Wl + cl' (per batch row vector)
    q_eff  = (x @ Wq + bq) @ Wt * lat_f = x @ (Wq@Wt * lat_f) + b_eff

so each core runs a plain causal attention over its 3 heads and writes the
partial (row-slice of c_proj) output projection; the host sums the 4
partials per batch and adds c_proj_b.

v6 design notes (vs the 130us v5), fast-mode HW exec ~114us:
  * PE warm-up: a stream of identity matmuls issued right after the
    framework preamble (~7us, fixed) keeps the PE HAM clock gate open
    through the input-DMA wait, so the first real qkv matmuls run at the
    full 2.4GHz (v5: first matmul at 12.4us, half clock until 21us).
  * input DMAs: one descriptor per tensor (3D APs), w/x split in jp
    halves so the first LDWEIGHTS goes as soon as half has landed; bias
    and wp ride the GpSimd software-DMA path (a leading bias DMA of 384
    4-byte packets at the head of the SP queue cost ~5us of startup).
  * V is produced directly in [keys, dims] layout by flipping the matmul
    orientation (stationary = x position-tile, moving = Wv slab): kills
    the 16 PE transposes, 32 DVE copies and the half-empty jm4 qkv slot.
    The v bias is folded into c_proj_b on the host (exact); the k bias
    is kept only because slot jm2 mixes q2|k2.
  * softmax reciprocal moved off ACT onto the DVE (copy-to-base-0 +
    reciprocal_approx_fast + mul; measured HW constraints: the custom op
    needs base partition 0 for both operands, dual-input DVE ops need
    equal input bases, single-input ops may shift). ACT only runs exps.
  * guard memsets moved from DVE to GpSimd.
  * tail: proj for the last two row-tiles splits into phaseA (heads 0/1,
    held in the then-idle sc psum pool) during attention(3) and phaseB
    after; the last head's normalize runs in two column halves so
    phaseB unblocks early; final evictions split ACT/DVE and the last
    transfers issue from both HWDGE queues.
  * fp16 everywhere on the PE (fp8 DoubleRow was evaluated: each e4m3
    quantization stage alone costs ~2-3e-2 max-rel error vs the 2e-2
    budget -- dead end).
  * NOTE: the chip drifts between a 2.4GHz fast mode (~114us) and a
    power-throttled ~2.0GHz mode (~136us); the mode persists for minutes
    and is not controlled by this kernel.
"""

import os
import sys

for _p in ("/opt/trn_rl_repo", "/opt/pypackages"):
    if os.path.isdir(_p) and _p not in sys.path:
        sys.path.insert(0, _p)

import numpy as np

S = 2048
D = 768
NH = 12
HD = 64
HPC = 3  # heads per core
N_CORES = 8
QC = 512  # q chunk (moving free dim)
NQC = S // QC  # 4
KT = 128  # key tile

# 64-wide column slots of the fused per-core q/k weight (768, 384):
SLOTS = [
    ("q", 0), ("q", 1),
    ("k", 0), ("k", 1),
    ("q", 2), ("k", 2),
]
K_CH = {0: 0, 1: 0, 2: 1}   # head -> qkvT_k chunk (rows: l0 0:64, l1/l2 64:128)

_CACHE = {}


def _build_program():
    import concourse.bass as bass
    from concourse import bacc, mybir
    from concourse.tile import TileContext
    from concourse.hw_specs import get_activation_tables

    f32 = mybir.dt.float32
    fp16 = mybir.dt.float16

    nc = bacc.Bacc("TRN2", target_bir_lowering=False, debug=False,
                   num_devices=N_CORES)

    x_d = nc.dram_tensor("xT", (D, S), fp16, kind="ExternalInput").ap()
    w_d = nc.dram_tensor("w_qkv", (D, 384), fp16, kind="ExternalInput").ap()
    wv_d = nc.dram_tensor("w_v", (D, 192), fp16, kind="ExternalInput").ap()
    b_d = nc.dram_tensor("b_qkv", (384,), f32, kind="ExternalInput").ap()
    wp_d = nc.dram_tensor("w_proj", (192, D), fp16, kind="ExternalInput").ap()
    out_d = nc.dram_tensor("out_partial", (S, D), fp16,
                           kind="ExternalOutput").ap()

    from contextlib import ExitStack

    with TileContext(nc) as tc, ExitStack() as ctx:
        # Pre-load the Exp activation table so the act-table pass never
        # inserts a swap mid-kernel.
        tabs = list(get_activation_tables(nc.m.arch).keys())
        nc.scalar.add_instruction(mybir.InstLoadActFuncSet(
            name=nc.get_next_instruction_name(),
            act_func_set_id=tabs.index("natural_log_exp_and_others"),
            ins=[], outs=[]))

        singles = ctx.enter_context(tc.tile_pool(name="singles", bufs=1))
        exp_pool = ctx.enter_context(tc.tile_pool(name="expT", bufs=4))
        rec_pool = ctx.enter_context(tc.tile_pool(name="rec", bufs=3))
        osb_pool = ctx.enter_context(tc.tile_pool(name="osb", bufs=4))
        sc_psum = ctx.enter_context(tc.tile_pool(name="sc_ps", bufs=2, space="PSUM"))
        pv_psum = ctx.enter_context(tc.tile_pool(name="pv_ps", bufs=2, space="PSUM"))
        mm_psum = ctx.enter_context(tc.tile_pool(name="mm_ps", bufs=2, space="PSUM"))

        w_sb = singles.tile([128, 6, 384], fp16)    # [jp slab, q/k slots]
        wv_sb = singles.tile([128, 6, 192], fp16)   # [jp slab, v slots]
        bias_sb = singles.tile([128, 3], f32)
        wp_sb = singles.tile([128, 2, D], fp16)
        xT = singles.tile([128, 6, S], fp16)

        # pad rows 64:128 of wp slab 1 with zeros: the proj's second
        # matmul then contracts a full 128 partitions (sub-128 contractions
        # run the PE at the unwarmed half clock)
        nc.vector.memset(wp_sb[64:128, 1, :], 0.0)

        # fp16 identity: now only PE warm-up fodder
        id16 = singles.tile([128, 128], fp16)
        nc.gpsimd.memset(id16, 0.0)
        nc.gpsimd.affine_select(
            out=id16, in_=id16, compare_op=mybir.AluOpType.not_equal,
            fill=1.0, base=0, pattern=[[-1, 128]], channel_multiplier=1)

        qkvT_k = singles.tile([128, 2, S], fp16)  # ch0 [k0|k1], ch1 [junk|k2]
        qk2 = singles.tile([128, 3, S], fp16)
        aT = singles.tile([128, 2, S], fp16)
        # [V_l | ones] stationaries: [128 keys, G, slab, (64 V|64 ones) x 3]
        vones = singles.tile([128, S // KT // 2, 2, HPC, 2, 64], fp16)

        # zero the padded/junk halves (NaN guards for the stationary
        # reads) on GpSimd, split per chunk so chunk c's pieces ride
        # the fill stream of attention(c-1).
        def emit_guards(c):
            cs = slice(c * QC, (c + 1) * QC)
            nc.gpsimd.memset(aT[64:128, 1, cs], 0.0)
            nc.gpsimd.memset(qkvT_k[0:64, 1, cs], 0.0)
            nc.gpsimd.memset(qk2[64:128, 0, cs], 0.0)
            nc.gpsimd.memset(qk2[0:64, 1, cs], 0.0)
            nc.gpsimd.memset(qk2[0:64, 2, cs], 0.0)

        def at_slice(l, fs):
            if l == 0:
                return aT[0:64, 0, fs]
            if l == 1:
                return aT[64:128, 0, fs]
            return aT[0:64, 1, fs]

        # Input DMAs. The first qkv group needs w + x chunk0: those are
        # split in jp halves so the first LDWEIGHTS can go as soon as the
        # first half lands. Small/late-needed tensors (bias, wp) ride the
        # GpSimd software-DMA path so their tiny packets never clog the
        # queues carrying w/x (a leading bias DMA cost ~5us of startup in
        # v6.0: 384 4-byte packets at the head of the SP queue).
        w_src = w_d.rearrange("(j p) e -> p j e", p=128)
        x_src = x_d.rearrange("(j p) s -> p j s", p=128)
        nc.sync.dma_start(out=w_sb[:, 0:3, :], in_=w_src[:, 0:3, :])
        nc.scalar.dma_start(out=xT[:, 0:3, 0:QC], in_=x_src[:, 0:3, 0:QC])
        nc.sync.dma_start(out=w_sb[:, 3:6, :], in_=w_src[:, 3:6, :])
        nc.scalar.dma_start(out=xT[:, 3:6, 0:QC], in_=x_src[:, 3:6, 0:QC])
        nc.gpsimd.dma_start(out=bias_sb,
                            in_=b_d.rearrange("(c p) -> p c", p=128))
        nc.sync.dma_start(out=wv_sb, in_=wv_d.rearrange("(j p) e -> p j e", p=128))
        nc.gpsimd.dma_start(out=wp_sb[:, 0, :], in_=wp_d[0:128, :])
        nc.gpsimd.dma_start(out=wp_sb[0:64, 1, :], in_=wp_d[128:192, :])
        # x chunk 1 separately: qkv(1) runs as attention(0) fill inside
        # the HAM warm-up window -- the earlier it can go, the denser the
        # cold region and the sooner the clock un-throttles.
        nc.scalar.dma_start(out=xT[:, :, QC:2 * QC], in_=x_src[:, :, QC:2 * QC])
        nc.scalar.dma_start(out=xT[:, :, 2 * QC:S], in_=x_src[:, :, 2 * QC:S])

        # PE warm-up: junk matmuls on the identity tile keep the HAM
        # activity window hot while the input DMAs land (first real matmul
        # can go at ~12.5us: preamble ~7us + w/x first halves ~5us), so
        # real matmuls start at the full 2.4GHz clock. The PE queue is
        # in-order, so these must ALL come before the first real matmul.
        wu_state = {"n": 0}

        def emit_wu(n=1):
            for _ in range(n):
                wps = mm_psum.tile([128, 128], f32, tag="mm",
                                   name=f"wu{wu_state['n']}")
                wu_state["n"] += 1
                nc.tensor.matmul(wps, id16, id16, start=True, stop=True)

        emit_wu(20)

        # ---- qkv q/k: one jm group (128 out cols, 6 contraction matmuls) ----
        def emit_qkv_jm(c, jm):
            cs = slice(c * QC, (c + 1) * QC)
            ps = mm_psum.tile([128, QC], f32, tag="mm")
            for jp in range(6):
                nc.tensor.matmul(
                    ps,
                    w_sb[:, jp, jm * 128:(jm + 1) * 128],
                    xT[:, jp, cs],
                    start=(jp == 0), stop=(jp == 5))
            if jm == 0:      # [q0 | q1]
                nc.vector.tensor_scalar_add(
                    out=qk2[0:64, 0, cs], in0=ps[0:64, :],
                    scalar1=bias_sb[0:64, 0:1])
                nc.vector.tensor_scalar_add(
                    out=qk2[64:128, 1, cs], in0=ps[64:128, :],
                    scalar1=bias_sb[64:128, 0:1])
            elif jm == 1:    # [k0 | k1] -> one merged 128-row eviction
                nc.vector.tensor_scalar_add(
                    out=qkvT_k[:, 0, cs], in0=ps,
                    scalar1=bias_sb[:, 1:2])
            else:            # [q2 | k2]; q2 shifts partitions
                nc.vector.tensor_scalar_add(
                    out=qk2[64:128, 2, cs], in0=ps[0:64, :],
                    scalar1=bias_sb[0:64, 2:3])
                nc.vector.tensor_scalar_add(
                    out=qkvT_k[64:128, 1, cs], in0=ps[64:128, :],
                    scalar1=bias_sb[64:128, 2:3])

        # ---- V for one key subtile (128 keys), flipped orientation ----
        # stationary = x position-tile, moving = Wv slab: psum comes out
        # as [128 positions(keys), 192 v-dims] and is evicted straight
        # into the vones layout (v bias folded into c_proj_b on host).
        def emit_vm(st):
            G, sl = st // 2, st % 2
            ps = mm_psum.tile([128, 192], f32, tag="mm")
            for jp in range(6):
                nc.tensor.matmul(
                    ps,
                    xT[:, jp, st * 128:(st + 1) * 128],
                    wv_sb[:, jp, :],
                    start=(jp == 0), stop=(jp == 5))
            nc.vector.tensor_copy(
                out=vones[:, G, sl, :, 0, :],
                in_=ps.rearrange("p (l v) -> p l v", v=64))

        # ---- attention for chunk c, with PE filler interleaved ----
        def emit_attention(c, fill_one):
            for l in range(HPC):
                kch = K_CH[l]
                pv = pv_psum.tile([128, QC], f32, tag="pv")
                n_groups = 2 * (c + 1)

                def q0_of(kt):
                    return max(0, 128 * (kt - 4 * c))

                def emit_qk(G):
                    sc = sc_psum.tile([128, 2, QC], f32, tag="sc",
                                      name=f"sc_{c}_{l}_{G}")
                    for jj in range(2):
                        kt = 2 * G + jj
                        q0 = q0_of(kt)
                        nc.tensor.matmul(
                            sc[:, jj, q0:QC],
                            qkvT_k[:, kch, kt * 128:(kt + 1) * 128],
                            qk2[:, l, c * QC + q0:(c + 1) * QC],
                            start=True, stop=True)
                    return sc

                def emit_exp_pv(G, sc):
                    diag = G >= 2 * c
                    q0e = q0_of(2 * G)
                    expT = exp_pool.tile([128, 2, QC], fp16, tag="expT",
                                         name=f"expT_{c}_{l}_{G}")
                    nc.scalar.activation(
                        out=expT[:, :, q0e:QC],
                        in_=sc[:, :, q0e:QC],
                        func=mybir.ActivationFunctionType.Exp,
                        scale=1.0 / np.sqrt(np.float32(HD)))
                    if diag:
                        # causal fill on the two diagonal-region slabs:
                        # even slab: keep where q >= key  (1 + t - p > 0)
                        nc.gpsimd.affine_select(
                            out=expT[:, 0, q0e:q0e + 128],
                            in_=expT[:, 0, q0e:q0e + 128],
                            compare_op=mybir.AluOpType.is_gt,
                            fill=0.0, base=1, pattern=[[1, 128]],
                            channel_multiplier=-1)
                        # odd slab: keep where q-128 >= key (t - 127 - p > 0);
                        # also zero-fills the [q0e, q0e+128) stale region
                        nc.gpsimd.affine_select(
                            out=expT[:, 1, q0e:q0e + 256],
                            in_=expT[:, 1, q0e:q0e + 256],
                            compare_op=mybir.AluOpType.is_gt,
                            fill=0.0, base=-127, pattern=[[1, 256]],
                            channel_multiplier=-1)
                    for jj in range(2):
                        kt = 2 * G + jj
                        q0 = q0_of(kt)
                        nc.tensor.matmul(
                            pv[:, q0:QC],
                            vones[:, G, jj, l, :, :].rearrange(
                                "p a b -> p (a b)"),
                            expT[:, jj, q0:QC],
                            start=(G == 0 and jj == 0),
                            stop=(G == n_groups - 1 and jj == 1))

                # one-deep software pipeline: QK(G+1) before exp/PV(G)
                prev = emit_qk(0)
                for G in range(1, n_groups):
                    sc = emit_qk(G)
                    emit_exp_pv(G - 1, prev)
                    fill_one()
                    prev = sc
                emit_exp_pv(n_groups - 1, prev)
                fill_one()
                # normalize, all on the DVE (measured HW constraints: the
                # custom recip op only works at base partition 0 for BOTH
                # operands; dual-input DVE ops need equal input bases;
                # single-input DVE ops may shift partitions freely):
                #   copy den (pv[64:128]) down to base 0, recip, mul.
                # The very last head's normalize gates the proj tail, so
                # it runs in two column halves: proj(12,13) unblock after
                # the first half's mul.
                rec = rec_pool.tile([128, QC], f32, tag="rec")
                rec2 = rec_pool.tile([128, QC], f32, tag="rec")
                halves = 2 if (c == NQC - 1 and l == HPC - 1) else 1
                hw = QC // halves
                for h in range(halves):
                    hs = slice(h * hw, (h + 1) * hw)
                    nc.vector.tensor_copy(out=rec[0:64, hs],
                                          in_=pv[64:128, hs])
                    nc.vector.reciprocal_approx_fast(
                        out=rec2[0:64, hs], in_=rec[0:64, hs])
                    nc.vector.tensor_mul(
                        out=at_slice(l, slice(c * QC + h * hw,
                                              c * QC + (h + 1) * hw)),
                        in0=pv[0:64, hs], in1=rec2[0:64, hs])

        # split-phase proj for the very last attention chunk: phase A (the
        # heads-0/1 matmuls, independent of head 2) runs in the final fill
        # slot while head 2's softmax normalize chain drains; phase B
        # finishes after the aT write.
        proj_hold = {}

        def emit_proj_phaseA(st, pool=None, tag="mm"):
            # pool override: the last phaseA's go into the sc psum pool,
            # which is idle once attention(3)'s final QK has run -- the
            # held tiles then don't block the mm rotation.
            pool = pool or mm_psum
            pos = []
            for nch in range(2):
                po = pool.tile([128, 384], f32, tag=tag,
                               name=f"poA_{st}_{nch}")
                nc.tensor.matmul(
                    po,
                    aT[:, 0, st * 128:(st + 1) * 128],
                    wp_sb[:, 0, nch * 384:(nch + 1) * 384],
                    start=True, stop=False)
                pos.append(po)
            proj_hold[st] = pos

        def emit_proj_phaseB(st):
            osb = osb_pool.tile([128, D], fp16, tag="osb")
            for nch, po in enumerate(proj_hold.pop(st)):
                nc.tensor.matmul(
                    po,
                    aT[:, 1, st * 128:(st + 1) * 128],
                    wp_sb[:, 1, nch * 384:(nch + 1) * 384],
                    start=False, stop=True)
                if nch == 0:
                    nc.scalar.copy(
                        out=osb[:, nch * 384:(nch + 1) * 384], in_=po)
                else:
                    nc.vector.tensor_copy(
                        out=osb[:, nch * 384:(nch + 1) * 384], in_=po)
                eng = nc.scalar if nch == 0 else nc.sync
                eng.dma_start(
                    out=out_d[st * 128:(st + 1) * 128,
                              nch * 384:(nch + 1) * 384],
                    in_=osb[:, nch * 384:(nch + 1) * 384])

        def emit_proj_st(st, tail=False):
            osb = osb_pool.tile([128, D], fp16, tag="osb")
            for nch in range(2):
                # at the tail, alternate po tiles between the mm and pv
                # psum pools (pv is done by then): 4 in-flight po tiles
                # instead of 2, so the matmul->evict->matmul ladder
                # pipelines instead of serializing on bank reuse.
                pool = pv_psum if (tail and nch == 1) else mm_psum
                po = pool.tile([128, 384], f32, tag="pv" if pool is pv_psum
                               else "mm")
                nc.tensor.matmul(
                    po,
                    aT[:, 0, st * 128:(st + 1) * 128],
                    wp_sb[:, 0, nch * 384:(nch + 1) * 384],
                    start=True, stop=False)
                nc.tensor.matmul(
                    po,
                    aT[:, 1, st * 128:(st + 1) * 128],
                    wp_sb[:, 1, nch * 384:(nch + 1) * 384],
                    start=False, stop=True)
                if tail and nch == 0:
                    # ACT is idle after the last exp: halve the tail's
                    # serial eviction chain
                    nc.scalar.copy(
                        out=osb[:, nch * 384:(nch + 1) * 384], in_=po)
                else:
                    nc.vector.tensor_copy(
                        out=osb[:, nch * 384:(nch + 1) * 384], in_=po)
                if tail:
                    # per-half DMA so the final transfer starts as soon as
                    # its half is evicted; alternate issue queues (the
                    # 0.6us descriptor generation otherwise serializes the
                    # last transfers on one queue)
                    eng = nc.scalar if nch == 0 else nc.sync
                    eng.dma_start(
                        out=out_d[st * 128:(st + 1) * 128,
                                  nch * 384:(nch + 1) * 384],
                        in_=osb[:, nch * 384:(nch + 1) * 384])
            if not tail:
                nc.sync.dma_start(out=out_d[st * 128:(st + 1) * 128, :],
                                  in_=osb)

        # ---- main schedule ----
        # Fill inventory per attention chunk. attention(3) has the largest
        # exp load (the PE idles waiting on the ACT engine there), so every
        # fill whose data dependencies allow it is deferred into att(3):
        # only the q evictions of qkv(3) (jm 0, 2) must precede att(3)
        # (its first QK reads chunk-3 queries); k/v of chunk 3 are first
        # read at group 6+, after the early fill slots have run.
        emit_guards(0)
        # vones "ones" halves: G0/G1 are read by attention(0); later G's
        # ride the fill streams (gpsimd queue order matters -- guards(0)
        # gate attention(0)'s first QK, so they go first).
        for G in range(2):
            nc.gpsimd.memset(vones[:, G, :, :, 1, :], 1.0)
        for jm in range(3):
            emit_qkv_jm(0, jm)
        # only vm0/vm1 must precede attention(0)'s first PV; vm2/vm3 ride
        # the fill stream (eagerly, before PV group 1), so att(0)'s first
        # QK/exp start ~3us earlier
        for st in range(2):
            emit_vm(st)

        # NOTE emission-order constraint: attention(c) reads vones tiles up
        # to st=4c+3, so vm(st) must be EMITTED before the attention chunk
        # that reads it (or eagerly at its start, for chunk 3).
        fill_plan = {
            0: [("v", 2), ("v", 3)] + [(1, jm) for jm in range(3)]
               + [("g", 1), ("o", 2), ("o", 3)]
               + [("v", st) for st in range(4, 8)],
            1: [("g", 2)] + [("o", G) for G in range(4, 8)]
               + [(2, jm) for jm in range(3)]
               + [("v", st) for st in range(8, 12)]
               + [("p", st) for st in range(0, 4)],
            2: [("g", 3), (3, 0), (3, 2)]
               + [("p", st) for st in range(4, 8)],
            3: [(3, 1)]
               + [("v", st) for st in range(12, 16)]
               + [("p", st) for st in range(8, 12)]
               + [("pA", 12), ("pAsc", 13)],
        }
        # chunk 3's qkv/v fills feed attention(3) itself (its groups 6+
        # read chunk-3 K/V): they must stay eager; only proj is paced.
        eager_n = {0: 5, 1: 0, 2: 0, 3: 5}

        def run_fill(f):
            if f[0] == "pAsc":
                emit_proj_phaseA(f[1], pool=sc_psum, tag="sc")
            elif f[0] == "pA":
                emit_proj_phaseA(f[1])
            elif f[0] == "v":
                emit_vm(f[1])
            elif f[0] == "p":
                emit_proj_st(f[1])
            elif f[0] == "g":
                emit_guards(f[1])
            elif f[0] == "o":
                nc.gpsimd.memset(vones[:, f[1], :, :, 1, :], 1.0)
            else:
                emit_qkv_jm(f[0], f[1])

        for c in range(NQC):
            fills = list(fill_plan[c])
            n_slots = HPC * 2 * (c + 1)
            state = {"slot": 0, "done": 0}
            ne = eager_n[c]

            def fill_one():
                state["slot"] += 1
                want = max(ne, len(fill_plan[c]) * state["slot"] // n_slots)
                while fills and state["done"] < want:
                    run_fill(fills.pop(0))
                    state["done"] += 1

            emit_attention(c, fill_one)
            while fills:
                run_fill(fills.pop(0))
        emit_proj_phaseB(12)
        emit_proj_phaseB(13)
        for st in range(14, 16):
            emit_proj_st(st, tail=True)

    nc.compile()
    return nc


def _fold_inputs(x, latent_syntax, c_attn_w, c_attn_b, c_proj_w, c_proj_b,
                 lmf_text_w, lmf_text_b, lmf_lat_w, lmf_lat_b):
    """Host-side algebraic folding of the LMF into per-core weights."""
    f = np.float32
    x = np.ascontiguousarray(x, dtype=f)
    B = x.shape[0]
    Wq, Wk, Wv = (c_attn_w[:, :D], c_attn_w[:, D:2 * D], c_attn_w[:, 2 * D:])
    bq, bk, bv = (c_attn_b[:D], c_attn_b[D:2 * D], c_attn_b[2 * D:])
    Wt = lmf_text_w.sum(0).astype(f)       # (D+1, D)
    ct = lmf_text_b.sum(0).astype(f)
    Wl = lmf_lat_w.sum(0).astype(f)
    cl = lmf_lat_b.sum(0).astype(f)
    W_text = (Wq.astype(f) @ Wt[:D])       # (D, D)
    b_text = bq.astype(f) @ Wt[:D] + Wt[D] + ct
    lat = latent_syntax[:, 0, :].astype(f)
    lat1 = np.concatenate([lat, np.ones((B, 1), f)], axis=-1)
    lat_f = lat1 @ Wl + cl                 # (B, D)

    in_maps = []
    for core in range(N_CORES):
        b = core // 4
        g = core % 4
        Wq_eff = W_text * lat_f[b][None, :]
        bq_eff = b_text * lat_f[b]
        mats = {"q": Wq_eff, "k": Wk.astype(f)}
        vecs = {"q": bq_eff, "k": bk.astype(f)}
        W_core = np.empty((D, 384), f)
        b_core = np.zeros((384,), f)
        for slot, (kind, l) in enumerate(SLOTS):
            h = 3 * g + l
            W_core[:, slot * 64:(slot + 1) * 64] = \
                mats[kind][:, h * 64:(h + 1) * 64]
            b_core[slot * 64:(slot + 1) * 64] = vecs[kind][h * 64:(h + 1) * 64]
        in_maps.append({
            "xT": np.ascontiguousarray(x[b].T.astype(np.float16)),
            "w_qkv": np.ascontiguousarray(W_core.astype(np.float16)),
            "w_v": np.ascontiguousarray(
                Wv[:, 192 * g:192 * (g + 1)].astype(np.float16)),
            "b_qkv": b_core,
            "w_proj": np.ascontiguousarray(
                c_proj_w[192 * g:192 * (g + 1), :].astype(np.float16)),
        })
    return in_maps


def _get_program():
    if "nc" not in _CACHE:
        _CACHE["nc"] = _build_program()
    return _CACHE["nc"]


def kernel(**inputs):
    from concourse import bass_utils

    nc = _get_program()
    in_maps = _fold_inputs(**inputs)
    res = bass_utils.run_bass_kernel_spmd(nc, in_maps,
                                          core_ids=list(range(N_CORES)))
    B = inputs["x"].shape[0]
    # v bias folded here: a_h += bv_h for every position, so the proj
    # output gains the constant row bv @ c_proj_w (exact).
    bv = inputs["c_attn_b"][2 * D:].astype(np.float32)
    cpb = inputs["c_proj_b"].astype(np.float32) + \
        bv @ inputs["c_proj_w"].astype(np.float32)
    out = np.zeros((B, S, D), np.float32)
    for b in range(B):
        acc = np.zeros((S, D), np.float32)
        for g in range(4):
            acc += res.results[4 * b + g]["out_partial"].astype(np.float32)
        out[b] = acc + cpb[None, :]
    return out
